# revision 3
# baseline (speedup 1.0000x reference)
"""Group VQ (vq_codebook) Trainium2 Bass kernel.

Strategy: data-parallel over batch B=16 across 8 cores (2 batches/core).
Per core, for each (group g, batch b, 125-token tile): compute scores
s[t,k] = 2*x·e_k - |e_k|^2 via fp32 matmul (token-stationary, codes moving)
into PSUM, then one DVE segmented reduce_max (1024 codes -> 16 segment
maxima) per tile. Segment maxima go to HBM; the host picks the winning
segment per token (exact comparison of device fp32 values) and rescores
its 64 codes in fp64 to recover the exact argmin, then gathers the code
vectors. Device work is one matmul pair + one DVE pass per tile.

e2 folding: host prepends a ones-row to each group's x slab (row 64) and
builds Etilde[g] = [2*E^T; -|e|^2] so one matmul yields the full score.
"""
import sys
import numpy as np
from contextlib import ExitStack

sys.path.insert(0, "/opt/trn_rl_repo")

B, C, F, T = 16, 2, 256, 4000
G, K, D = 8, 1024, 64
NCORES = 8
NB = B // NCORES          # batches per core = 2
TT = 125                  # tokens per tile (4000 = 32*125)
ST = 500                  # tokens per x-DMA supertile (4 tiles)
NTILES = T // TT          # 32
NSUP = T // ST            # 8
NSEG = 16                 # segments per 1024 codes
SEGW = K // NSEG          # 64 codes per segment

_compiled = None


def _build_program():
    import concourse.bass as bass
    import concourse.tile as tile
    from concourse import bacc, mybir

    nc = bacc.Bacc(
        "TRN2",
        target_bir_lowering=False,
        debug=False,
        enable_asserts=False,
        num_devices=NCORES,
    )
    f32 = mybir.dt.float32
    xa = nc.dram_tensor("xa", [NB, G, 65, T], f32, kind="ExternalInput").ap()
    et = nc.dram_tensor("et", [G, 65, K], f32, kind="ExternalInput").ap()
    om = nc.dram_tensor(
        "om", [G * NB, TT, NTILES * NSEG], f32, kind="ExternalOutput"
    ).ap()

    with tile.TileContext(nc) as tc, ExitStack() as ctx:
        epool = ctx.enter_context(tc.tile_pool(name="e", bufs=1))
        xpool = ctx.enter_context(tc.tile_pool(name="x", bufs=4))
        ppool = ctx.enter_context(
            tc.tile_pool(name="ps", bufs=3, space=bass.MemorySpace.PSUM)
        )
        mpool = ctx.enter_context(tc.tile_pool(name="mseg", bufs=2))

        etiles = []
        for g in range(G):
            e_t = epool.tile([65, K], f32, tag=f"e{g}")
            nc.sync.dma_start(e_t[:], et[g])
            etiles.append(e_t)

        for g in range(G):
            for b in range(NB):
                m_sb = mpool.tile([TT, NTILES * NSEG], f32)
                for s in range(NSUP):
                    xt = xpool.tile([65, ST], f32)
                    nc.sync.dma_start(xt[:], xa[b, g, :, s * ST:(s + 1) * ST])
                    for k4 in range(4):
                        tloc = s * 4 + k4
                        ps = ppool.tile([TT, K], f32)
                        lhs = xt[:, k4 * TT:(k4 + 1) * TT]
                        nc.tensor.matmul(
                            ps[:, 0:512], lhs,
                            etiles[g][:, 0:512],
                            start=True, stop=True,
                        )
                        nc.tensor.matmul(
                            ps[:, 512:1024], lhs,
                            etiles[g][:, 512:1024],
                            start=True, stop=True,
                        )
                        # segmented max: [TT, NSEG, SEGW] -> [TT, NSEG]
                        nc.vector.tensor_reduce(
                            m_sb[:, tloc * NSEG:(tloc + 1) * NSEG],
                            ps[:].rearrange("p (s w) -> p s w", s=NSEG, w=SEGW),
                            axis=mybir.AxisListType.X,
                            op=mybir.AluOpType.max,
                        )
                nc.sync.dma_start(om[g * NB + b], m_sb[:])

    nc.compile()
    return nc


def _get_compiled():
    global _compiled
    if _compiled is None:
        _compiled = _build_program()
    return _compiled


def _prep_inputs(x, codebooks):
    # x: [B,C,F,T] fp32 -> per-core xa [NB, G, 65, T] with ones row 64
    xg = np.ascontiguousarray(x.reshape(B, G, D, T))
    ones = np.ones((B, G, 1, T), dtype=np.float32)
    xa_full = np.concatenate([xg, ones], axis=2)  # [B, G, 65, T]
    # Etilde: [G, 65, K] : rows 0..63 = 2*E^T, row 64 = -|e|^2
    et = np.empty((G, 65, K), dtype=np.float32)
    et[:, :64, :] = 2.0 * np.transpose(codebooks, (0, 2, 1))
    et[:, 64, :] = -(codebooks.astype(np.float32) ** 2).sum(-1)
    return xa_full, et


def run_device(x, codebooks, trace=False):
    from concourse.bass_utils import run_bass_kernel_spmd

    nc = _get_compiled()
    xa_full, et = _prep_inputs(np.asarray(x, np.float32),
                               np.asarray(codebooks, np.float32))
    in_maps = []
    for core in range(NCORES):
        xa_c = np.ascontiguousarray(xa_full[core * NB:(core + 1) * NB])
        in_maps.append({"xa": xa_c, "et": et})
    res = run_bass_kernel_spmd(nc, in_maps, list(range(NCORES)), trace=trace)
    return res


def _host_finish(x, codebooks, seg_best):
    """seg_best: [G, B, T] int winning segment per token.
    Rescore that segment's 64 codes in fp64 -> exact argmin -> gather."""
    xg = x.reshape(B, G, D, T)
    out = np.empty((B, G, D, T), dtype=np.float32)
    for g in range(G):
        cb = codebooks[g]                       # [K, D]
        cb64 = cb.astype(np.float64)
        e2 = (cb64 * cb64).sum(-1)              # [K]
        for b in range(B):
            tok = xg[b, g].T.astype(np.float64)     # [T, D]
            seg = seg_best[g, b]                    # [T]
            cand = seg[:, None] * SEGW + np.arange(SEGW)[None, :]  # [T, 64]
            ecand = cb64[cand]                      # [T, 64, D]
            scores = 2.0 * np.einsum('td,tkd->tk', tok, ecand) - e2[cand]
            idx = cand[np.arange(T), np.argmax(scores, axis=1)]
            out[b, g] = cb[idx].T                   # [D, T]
    return out.reshape(B, C, F, T)


def kernel(x, codebooks):
    x = np.asarray(x, dtype=np.float32)
    codebooks = np.asarray(codebooks, dtype=np.float32)
    res = run_device(x, codebooks)
    # om [G*NB, TT, NTILES*NSEG] ; token t = tloc*TT + p
    m16 = np.empty((G, B, T, NSEG), dtype=np.float32)
    for core in range(NCORES):
        o = res.results[core]["om"].reshape(G, NB, TT, NTILES, NSEG)
        m16[:, core * NB:(core + 1) * NB] = o.transpose(0, 1, 3, 2, 4).reshape(
            G, NB, T, NSEG
        )
    seg_best = np.argmax(m16, axis=-1)          # [G, B, T]
    q = _host_finish(x, codebooks, seg_best)
    x_q = x + (q - x)
    return x_q, q


# revision 9
# speedup vs baseline: 1.2382x; 1.2382x over previous
"""Group VQ (vq_codebook) Trainium2 Bass kernel.

Strategy: data-parallel over batch B=16 across 8 cores (2 batches/core).
Per core, for each (group g, batch b, 125-token tile): compute scores
s[t,k] = 2*x·e_k - |e_k|^2 via fp32 matmul (token-stationary, codes moving)
into PSUM, then one DVE segmented reduce_max (1024 codes -> 16 segment
maxima) per tile. Segment maxima go to HBM; the host picks the winning
segment per token (exact comparison of device fp32 values) and rescores
its 64 codes in fp64 to recover the exact argmin, then gathers the code
vectors. Device work is one matmul pair + one DVE pass per tile.

e2 folding: host prepends a ones-row to each group's x slab (row 64) and
builds Etilde[g] = [2*E^T; -|e|^2] so one matmul yields the full score.
"""
import sys
import numpy as np
from contextlib import ExitStack

sys.path.insert(0, "/opt/trn_rl_repo")

B, C, F, T = 16, 2, 256, 4000
G, K, D = 8, 1024, 64
NCORES = 8
NB = B // NCORES          # batches per core = 2
TT = 125                  # tokens per tile (4000 = 32*125)
ST = 500                  # tokens per x-DMA supertile (4 tiles)
NTILES = T // TT          # 32
NSUP = T // ST            # 8
NSEG = 16                 # segments per 1024 codes
SEGW = K // NSEG          # 64 codes per segment

_compiled = None


def _build_program():
    import concourse.bass as bass
    import concourse.tile as tile
    from concourse import bacc, mybir

    nc = bacc.Bacc(
        "TRN2",
        target_bir_lowering=False,
        debug=False,
        enable_asserts=False,
        num_devices=NCORES,
    )
    f32 = mybir.dt.float32
    f16 = mybir.dt.float16
    # x and Etilde each split into 2 fp16 terms (hi/lo); the three cross
    # products xh*eh + xh*el + xl*eh recover fp32 accuracy (~2^-22).
    xah = nc.dram_tensor("xah", [NB, G, 65, T], f16, kind="ExternalInput").ap()
    xal = nc.dram_tensor("xal", [NB, G, 65, T], f16, kind="ExternalInput").ap()
    eth = nc.dram_tensor("eth", [G, 65, K], f16, kind="ExternalInput").ap()
    etl = nc.dram_tensor("etl", [G, 65, K], f16, kind="ExternalInput").ap()
    om = nc.dram_tensor(
        "om", [G * NB, TT, NTILES * NSEG], f32, kind="ExternalOutput"
    ).ap()

    with tile.TileContext(nc) as tc, ExitStack() as ctx:
        epool = ctx.enter_context(tc.tile_pool(name="e", bufs=1))
        xpool = ctx.enter_context(tc.tile_pool(name="x", bufs=4))
        ppool = ctx.enter_context(
            tc.tile_pool(name="ps", bufs=3, space=bass.MemorySpace.PSUM)
        )
        mpool = ctx.enter_context(tc.tile_pool(name="mseg", bufs=2))

        etiles = []
        for g in range(G):
            duo = []
            for nm, src in (("h", eth), ("l", etl)):
                e_t = epool.tile([65, K], f16, tag=f"e{nm}{g}")
                nc.sync.dma_start(e_t[:], src[g])
                duo.append(e_t)
            etiles.append(duo)

        for g in range(G):
            for b in range(NB):
                m_sb = mpool.tile([TT, NTILES * NSEG], f32)
                for s in range(NSUP):
                    xth = xpool.tile([65, ST], f16, tag="xh")
                    nc.sync.dma_start(xth[:], xah[b, g, :, s * ST:(s + 1) * ST])
                    xtl = xpool.tile([65, ST], f16, tag="xl")
                    nc.sync.dma_start(xtl[:], xal[b, g, :, s * ST:(s + 1) * ST])
                    for k4 in range(4):
                        tloc = s * 4 + k4
                        ps = ppool.tile([TT, K], f32)
                        sl = slice(k4 * TT, (k4 + 1) * TT)
                        eh, el = etiles[g]
                        for c0 in (0, 512):
                            cs = slice(c0, c0 + 512)
                            nc.tensor.matmul(ps[:, cs], xth[:, sl], eh[:, cs],
                                             start=True, stop=False)
                            nc.tensor.matmul(ps[:, cs], xth[:, sl], el[:, cs],
                                             start=False, stop=False)
                            nc.tensor.matmul(ps[:, cs], xtl[:, sl], eh[:, cs],
                                             start=False, stop=True)
                        # segmented max: [TT, NSEG, SEGW] -> [TT, NSEG]
                        nc.vector.tensor_reduce(
                            m_sb[:, tloc * NSEG:(tloc + 1) * NSEG],
                            ps[:].rearrange("p (s w) -> p s w", s=NSEG, w=SEGW),
                            axis=mybir.AxisListType.X,
                            op=mybir.AluOpType.max,
                        )
                nc.sync.dma_start(om[g * NB + b], m_sb[:])

    nc.compile()
    return nc


def _get_compiled():
    global _compiled
    if _compiled is None:
        _compiled = _build_program()
    return _compiled


def _prep_inputs(x, codebooks):
    # x: [B,C,F,T] fp32 -> per-core xa [B, G, 65, T] with ones row 64,
    # split into fp16 hi/lo pairs.
    xg = np.ascontiguousarray(x.reshape(B, G, D, T))
    ones = np.ones((B, G, 1, T), dtype=np.float32)
    xa_full = np.concatenate([xg, ones], axis=2)  # [B, G, 65, T]
    xah = xa_full.astype(np.float16)
    xal = (xa_full - xah.astype(np.float32)).astype(np.float16)
    # Etilde: [G, 65, K] : rows 0..63 = 2*E^T, row 64 = -|e|^2
    et = np.empty((G, 65, K), dtype=np.float32)
    et[:, :64, :] = 2.0 * np.transpose(codebooks, (0, 2, 1))
    et[:, 64, :] = -(codebooks.astype(np.float32) ** 2).sum(-1)
    eth = et.astype(np.float16)
    etl = (et - eth.astype(np.float32)).astype(np.float16)
    return (xah, xal), (eth, etl)


def run_device(x, codebooks, trace=False):
    from concourse.bass_utils import run_bass_kernel_spmd

    nc = _get_compiled()
    (xah, xal), (eth, etl) = _prep_inputs(np.asarray(x, np.float32),
                                          np.asarray(codebooks, np.float32))
    in_maps = []
    for core in range(NCORES):
        sl = slice(core * NB, (core + 1) * NB)
        in_maps.append({"xah": np.ascontiguousarray(xah[sl]),
                        "xal": np.ascontiguousarray(xal[sl]),
                        "eth": eth, "etl": etl})
    res = run_bass_kernel_spmd(nc, in_maps, list(range(NCORES)), trace=trace)
    return res


def _host_finish(x, codebooks, seg_best):
    """seg_best: [G, B, T] int winning segment per token.
    Rescore that segment's 64 codes in fp64 -> exact argmin -> gather."""
    xg = x.reshape(B, G, D, T)
    out = np.empty((B, G, D, T), dtype=np.float32)
    for g in range(G):
        cb = codebooks[g]                       # [K, D]
        cb64 = cb.astype(np.float64)
        e2 = (cb64 * cb64).sum(-1)              # [K]
        for b in range(B):
            tok = xg[b, g].T.astype(np.float64)     # [T, D]
            seg = seg_best[g, b]                    # [T]
            cand = seg[:, None] * SEGW + np.arange(SEGW)[None, :]  # [T, 64]
            ecand = cb64[cand]                      # [T, 64, D]
            scores = 2.0 * np.einsum('td,tkd->tk', tok, ecand) - e2[cand]
            idx = cand[np.arange(T), np.argmax(scores, axis=1)]
            out[b, g] = cb[idx].T                   # [D, T]
    return out.reshape(B, C, F, T)


def kernel(x, codebooks):
    x = np.asarray(x, dtype=np.float32)
    codebooks = np.asarray(codebooks, dtype=np.float32)
    res = run_device(x, codebooks)
    # om [G*NB, TT, NTILES*NSEG] ; token t = tloc*TT + p
    m16 = np.empty((G, B, T, NSEG), dtype=np.float32)
    for core in range(NCORES):
        o = res.results[core]["om"].reshape(G, NB, TT, NTILES, NSEG)
        m16[:, core * NB:(core + 1) * NB] = o.transpose(0, 1, 3, 2, 4).reshape(
            G, NB, T, NSEG
        )
    seg_best = np.argmax(m16, axis=-1)          # [G, B, T]
    q = _host_finish(x, codebooks, seg_best)
    x_q = x + (q - x)
    return x_q, q
